# revision 11
# baseline (speedup 1.0000x reference)
"""Trainium2 Bass kernel for BinarizeConv2d block:
   y = round(2*clip(BN(conv3x3(x, sign(w))), -1, 1))/2

Output-channel sharding: each of 8 cores computes 4 output channels for ALL
16 images, so BN batch stats are fully local and NO collective is needed
(a collective would couple every core's NEFF span to the slowest core's
host->device staging, which dominates single-dispatch time).

Conv: x is shipped once as fp16 (exact products with +-1 weights; rel err
~1e-2 vs 2e-2 budget). 16 images run as 4 partition-lanes (g) x 4 batches.
K=32 (ci) matmuls on 16 concurrent 32x32 PE tiles: rows=32g (image lane),
cols=32j. Each col tile packs 28 row-pairs x 4 channels via zero-padded
weight columns: chain (j,k) has its 4 real weight cols at 4k+m and zeros
elsewhere, so every matmul writes the full 32-col group; the first chain
clears (start=True), later chains accumulate zeros harmlessly. PSUM comes
out dense -> full-width ACT drains, no repack.

Epilogue: bn_stats over the dense y_raw, cross-partition combine via tiny
fp32 sel matmuls, Newton-polished rsqrt, ACT affine + DVE/GPSIMD magic-number
round/clip to bf16 {-2..2}; host multiplies by 0.5 and concatenates the
8 cores' channel groups.
"""
import sys
sys.path.insert(0, "/opt/trn_rl_repo")
import numpy as np
import ml_dtypes
import concourse.bass as bass
import concourse.bacc as bacc
import concourse.tile as tile
from concourse import mybir
from concourse.bass_utils import run_bass_kernel_spmd

F32 = mybir.dt.float32
F16 = mybir.dt.float16
BF16 = mybir.dt.bfloat16

N_CORES = 8
CO_PC = 4         # output channels per core
C = 32
H = W = 224
WP = 226          # padded width
NB = 4            # image batches (4 lanes each)
NS = 4            # 56-row supers per image
SR = 56           # rows per super
SLOTS = 58        # input rows per super window (56 + 2 halo)
MAGIC = 12582912.0  # 1.5 * 2**23 -> fp32 round-to-nearest-even trick
EPS = 1e-5
NTOT = float(16 * H * W)  # elements per channel (all on one core)
HWs = H * W

_cache = {}


def _build_nc(loop_n=1, skip=(), dbg=False):
    nc = bacc.Bacc("TRN2", target_bir_lowering=False, debug=False,
                   num_devices=N_CORES)
    xs_ext = nc.declare_dram_parameter("xs", [16, C, H, W], F16, isOutput=False)
    dbg_ext = (nc.declare_dram_parameter("dbg", [128, NB, NS, 4, 448], F32,
                                         isOutput=True) if dbg else None)
    dbg2_ext = (nc.declare_dram_parameter("dbg2", [128, 8], F32,
                                          isOutput=True) if dbg == 2 else None)
    s_ext = nc.declare_dram_parameter("s", [128, 9, 7, 32], F16, isOutput=False)
    sel1_ext = nc.declare_dram_parameter("sel1", [128, CO_PC], F32,
                                         isOutput=False)
    sel2_ext = nc.declare_dram_parameter("sel2", [CO_PC, 128], F32,
                                         isOutput=False)
    g_ext = nc.declare_dram_parameter("g", [CO_PC, 1], F32, isOutput=False)
    b_ext = nc.declare_dram_parameter("b", [CO_PC, 1], F32, isOutput=False)
    y_ext = nc.declare_dram_parameter("y", [16, CO_PC, H, W], BF16,
                                      isOutput=True)

    with tile.TileContext(nc) as tc:
        with (
            tc.tile_pool(name="big", bufs=1) as big,
            tc.tile_pool(name="small", bufs=1) as small,
            tc.tile_pool(name="ph2", bufs=2) as ph2,
            tc.tile_pool(name="psum", bufs=1, space="PSUM") as psum,
        ):
            # x super chunk: partition p = 32g + ci ; free = (slot, WP)
            xb = [big.tile([128, SLOTS, WP], F16, name=f"xb{i}", tag=f"x{i}")
                  for i in range(2)]
            # dense conv out: partition p = 32j + 4k + m ; free=(b, s, g, i*w)
            y_raw = big.tile([128, NB, NS, 4, 448], F32)
            s_sb = small.tile([128, 9, 7, 32], F16)
            stats_buf = small.tile([128, NB * NS * 4, 6], F32)
            sel1_sb = small.tile([128, CO_PC], F32)
            sel2_sb = small.tile([CO_PC, 128], F32)
            g_sb = small.tile([CO_PC, 1], F32)
            b_sb = small.tile([CO_PC, 1], F32)
            stats_sq = small.tile([128, 2], F32)
            msq_scr = small.tile([128, NB * NS * 4 * 2], F32)
            red = small.tile([128, 4], F32)
            t4 = small.tile([CO_PC, 2], F32)
            fin = small.tile([CO_PC, 8], F32)
            sb4 = small.tile([CO_PC, 2], F32)
            ab128 = small.tile([128, 2], F32)

            psum_t = psum.tile([128, 8, 512], F32)

            for b_ in xb:
                nc.vector.memset(b_[:], 0.0)
            nc.vector.memset(stats_buf[:], 0.0)
            nc.sync.dma_start(out=s_sb[:], in_=s_ext[:])
            nc.sync.dma_start(out=sel1_sb[:], in_=sel1_ext[:])
            nc.sync.dma_start(out=sel2_sb[:], in_=sel2_ext[:])
            nc.sync.dma_start(out=g_sb[:], in_=g_ext[:])
            nc.sync.dma_start(out=b_sb[:], in_=b_ext[:])

            pfull = psum_t[:]
            pstride = pfull.ap[0][0]

            import contextlib
            loop_cm = (tc.For_i(0, loop_n, 1) if loop_n > 1
                       else contextlib.nullcontext())
            with loop_cm:
                _body(nc, tc, locals())
    nc.compile()
    return nc


def _body(nc, tc, env):
    xb = env["xb"]
    y_raw, s_sb = env["y_raw"], env["s_sb"]
    stats_buf, sel1_sb, sel2_sb = (env["stats_buf"], env["sel1_sb"],
                                   env["sel2_sb"])
    g_sb, b_sb = env["g_sb"], env["b_sb"]
    stats_sq, msq_scr, red = env["stats_sq"], env["msq_scr"], env["red"]
    t4, fin, sb4, ab128 = env["t4"], env["fin"], env["sb4"], env["ab128"]
    psum_t, ph2 = env["psum_t"], env["ph2"]
    y_ext, xs_ext = env["y_ext"], env["xs_ext"]
    pfull, pstride = env["pfull"], env["pstride"]
    skip = env["skip"]

    xap = xs_ext.ap()
    yap = y_ext.ap()

    # ---- phase 1: conv per (batch, super) ----
    for b in range(NB):
        for s in range(NS):
            idx = b * NS + s
            x_c = xb[idx % 2]
            # input rows 56s-1 .. 56s+56 -> slots 0..57
            if "xdma" not in skip:
                if s == 0:
                    nc.vector.memset(x_c[:, 0:1, :], 0.0)
                    r0, sl0, nrows = 0, 1, 57
                elif s == NS - 1:
                    nc.vector.memset(x_c[:, SLOTS - 1:SLOTS, :], 0.0)
                    r0, sl0, nrows = 56 * s - 1, 0, 57
                else:
                    r0, sl0, nrows = 56 * s - 1, 0, 58
                src = bass.AP(
                    tensor=xap.tensor,
                    offset=xap.offset + 4 * b * C * HWs + r0 * W,
                    ap=[[C * HWs, 4], [HWs, C], [W, nrows], [1, W]])
                nc.sync.dma_start(
                    out=x_c[:, sl0:sl0 + nrows, 1:225], in_=src)
            xv = x_c.rearrange("p r w -> p (r w)")
            bank0 = 4 * (idx % 2)
            for k in range(7 if "mm" not in skip else 0):
                for t in range(9):
                    kh, kw = divmod(t, 3)
                    for j in range(4):
                        off = (2 * (7 * j + k) + kh) * WP + kw
                        for g in range(4):
                            nc.tensor.matmul(
                                psum_t[32 * j:32 * j + 32, bank0 + g, 0:450],
                                s_sb[32 * g:32 * g + 32, t, k, :],
                                xv[32 * g:32 * g + 32, off:off + 450],
                                start=(k == 0 and t == 0),
                                stop=(k == 6 and t == 8),
                                tile_position=(32 * g, 32 * j))
            if "epi" in skip:
                continue
            for g in range(4):
                src = bass.AP(
                    tensor=pfull.tensor,
                    offset=pfull.offset + (bank0 + g) * 512,
                    ap=[[pstride, 124], [226, 2], [1, 224]])
                nc.scalar.copy(y_raw[0:124, b, s, g, :], src)
                if "stats" not in skip:
                    nc.vector.bn_stats(
                        out=stats_buf[0:124, idx * 4 + g, :],
                        in_=y_raw[0:124, b, s, g, :])

    if env.get("dbg_ext") is not None:
        nc.sync.dma_start(out=env["dbg_ext"].ap(), in_=y_raw[:])

    if "stats" in skip:
        return

    # ---- combine bn_stats chunks -> per-partition (sum, sumsq) [128,2] ----
    nslc = NB * NS * 4
    stats_fl = stats_buf.rearrange("p s (e t) -> p (s e) t", e=2, t=3)
    means = stats_fl[:, :, 1]
    ctv = stats_fl[:, :, 2]
    nc.vector.tensor_reduce(red[:, 0:1], means, mybir.AxisListType.X,
                            mybir.AluOpType.add)
    nc.vector.tensor_tensor(msq_scr[:], means, means, mybir.AluOpType.mult)
    nc.vector.tensor_reduce(red[:, 1:2], msq_scr[:], mybir.AxisListType.X,
                            mybir.AluOpType.add)
    nc.vector.tensor_reduce(red[:, 2:3], ctv, mybir.AxisListType.X,
                            mybir.AluOpType.add)
    nc.vector.tensor_scalar_mul(stats_sq[:, 0:1], red[:, 0:1], 224.0)
    nc.vector.tensor_scalar_mul(red[:, 3:4], red[:, 1:2], 224.0)
    nc.vector.tensor_tensor(stats_sq[:, 1:2], red[:, 3:4], red[:, 2:3],
                            mybir.AluOpType.add)

    # ---- combine (j,k) lanes: [128,2] -> [4,2] via PE ----
    nc.tensor.matmul(psum_t[0:CO_PC, 0, 0:2], sel1_sb[:], stats_sq[:],
                     start=True, stop=True)
    nc.scalar.copy(t4[:], psum_t[0:CO_PC, 0, 0:2])

    # ---- finalize per-channel scale/shift on partitions 0..3 ----
    mean = fin[:, 0:1]
    msqm = fin[:, 1:2]
    v = fin[:, 2:3]
    rec = fin[:, 3:4]
    a_ = fin[:, 4:5]
    bq = fin[:, 5:6]
    cq = fin[:, 6:7]
    sc = fin[:, 7:8]
    inv_n = float(np.float32(1.0) / np.float32(NTOT))
    nc.vector.tensor_scalar_mul(mean, t4[:, 0:1], inv_n)
    nc.vector.tensor_scalar_mul(msqm, t4[:, 1:2], inv_n)
    nc.vector.tensor_tensor(v, mean, mean, mybir.AluOpType.mult)
    nc.vector.tensor_tensor(v, msqm, v, mybir.AluOpType.subtract)
    nc.vector.tensor_scalar_add(v, v, EPS)
    nc.scalar.activation(rec, v, mybir.ActivationFunctionType.Sqrt)
    nc.vector.reciprocal(rec, rec)
    for _ in range(2):  # Newton polish: rec *= 1.5 - 0.5*v*rec^2
        nc.vector.tensor_tensor(a_, rec, rec, mybir.AluOpType.mult)
        nc.vector.tensor_tensor(bq, v, a_, mybir.AluOpType.mult)
        nc.vector.tensor_scalar(cq, bq, -0.5, 1.5, mybir.AluOpType.mult,
                                mybir.AluOpType.add)
        nc.vector.tensor_tensor(rec, rec, cq, mybir.AluOpType.mult)
    nc.vector.tensor_tensor(sc, g_sb[:], rec, mybir.AluOpType.mult)
    nc.vector.tensor_scalar_mul(sb4[:, 0:1], sc, 2.0)
    nc.vector.tensor_tensor(a_, mean, sc, mybir.AluOpType.mult)
    nc.vector.tensor_tensor(bq, b_sb[:], a_, mybir.AluOpType.subtract)
    nc.vector.tensor_scalar_mul(sb4[:, 1:2], bq, 2.0)

    # broadcast [4,2] -> [128,2]
    nc.tensor.matmul(psum_t[:, 1, 0:2], sel2_sb[:], sb4[:],
                     start=True, stop=True)
    nc.scalar.copy(ab128[:], psum_t[:, 1, 0:2])

    if env.get("dbg2_ext") is not None:
        dscr = env["small"].tile([128, 8], F32, name="dscr")
        nc.vector.memset(dscr[:], 0.0)
        nc.scalar.copy(dscr[:, 0:2], stats_sq[:])
        nc.scalar.copy(dscr[:, 2:4], ab128[:])
        nc.scalar.copy(dscr[0:CO_PC, 4:6], t4[:])
        nc.scalar.copy(dscr[0:CO_PC, 6:8], sb4[:])
        nc.sync.dma_start(out=env["dbg2_ext"].ap(), in_=dscr[:])

    # ---- phase 2: normalize + quantize + writeback, per (b, s) chunk ----
    for b in range(NB):
        for s in range(NS):
            if "ph2" in skip:
                break
            zin = y_raw[0:124, b, s].rearrange("p g w -> p (g w)")
            u = ph2.tile([128, 4 * 448], F32, tag="u")
            nc.scalar.activation(u[0:124], zin,
                                 mybir.ActivationFunctionType.Identity,
                                 bias=ab128[0:124, 1:2],
                                 scale=ab128[0:124, 0:1])
            u2 = ph2.tile([128, 4 * 448], F32, tag="u2")
            nc.gpsimd.tensor_scalar(u2[0:124], u[0:124], MAGIC, MAGIC + 2.0,
                                    mybir.AluOpType.add,
                                    mybir.AluOpType.min)
            o = ph2.tile([128, 4 * 448], BF16, tag="o")
            nc.vector.tensor_scalar(o[0:124], u2[0:124], MAGIC - 2.0, MAGIC,
                                    mybir.AluOpType.max,
                                    mybir.AluOpType.subtract)
            ov = o
            for g in range(4):
                for j in range(4):
                    dst = bass.AP(
                        tensor=yap.tensor,
                        offset=(yap.offset + (4 * b + g) * CO_PC * HWs
                                + (56 * s + 14 * j) * W),
                        ap=[[2 * W, 7], [HWs, CO_PC], [1, 448]])
                    nc.sync.dma_start(
                        out=dst,
                        in_=ov[32 * j:32 * j + 28, g * 448:(g + 1) * 448])


def _get_nc(**kw):
    kw.pop("collective", None)  # compat with old test harness
    key = tuple(sorted((k, tuple(v) if isinstance(v, (list, tuple, set)) else v)
                       for k, v in kw.items()))
    if key not in _cache:
        _cache[key] = _build_nc(**kw)
    return _cache[key]


def _host_consts(weight):
    w_bin = np.where(np.asarray(weight, dtype=np.float32) >= 0, 1.0,
                     -1.0).astype(np.float32)
    # sel over partitions p = 32j + col, col = 4k + m valid when col < 28
    p = np.arange(128)
    col = p % 32
    valid = col < 28
    m_of_p = col % 4
    sel1 = ((m_of_p[:, None] == np.arange(CO_PC)[None, :]) & valid[:, None]
            ).astype(np.float32)
    sel2 = np.ascontiguousarray(sel1.T)
    return w_bin, sel1, sel2


def _stage_s(w_bin, c):
    # S[32g+ci, t, k, 4k+m] = w_bin[4c+m, ci, t]
    wt = np.transpose(w_bin[4 * c:4 * c + 4].reshape(CO_PC, C, 9),
                      (1, 2, 0))  # [ci, t, m]
    s32 = np.zeros((C, 9, 7, 32), dtype=ml_dtypes.float16
                   if hasattr(ml_dtypes, "float16") else np.float16)
    for k in range(7):
        s32[:, :, k, 4 * k:4 * k + 4] = wt
    return np.tile(s32, (4, 1, 1, 1))  # [128, 9, 7, 32]


def make_in_maps(x, weight, gamma, beta):
    xs = np.asarray(x, dtype=np.float32).astype(np.float16)
    w_bin, sel1, sel2 = _host_consts(weight)
    gam = np.asarray(gamma, dtype=np.float32)
    bet = np.asarray(beta, dtype=np.float32)
    in_maps = []
    for c in range(N_CORES):
        in_maps.append({
            "xs": xs, "s": _stage_s(w_bin, c), "sel1": sel1, "sel2": sel2,
            "g": gam[4 * c:4 * c + 4].reshape(CO_PC, 1),
            "b": bet[4 * c:4 * c + 4].reshape(CO_PC, 1)})
    return in_maps


def kernel(x, weight, gamma, beta):
    nc = _get_nc()
    in_maps = make_in_maps(x, weight, gamma, beta)
    res = run_bass_kernel_spmd(nc, in_maps, list(range(N_CORES)))
    out = np.concatenate([res.results[c]["y"] for c in range(N_CORES)], axis=1)
    return out.astype(np.float32) * 0.5


# revision 20
# speedup vs baseline: 1.2929x; 1.2929x over previous
"""Trainium2 Bass kernel for BinarizeConv2d block:
   y = round(2*clip(BN(conv3x3(x, sign(w))), -1, 1))/2

Output-channel sharding: each of 8 cores computes 4 output channels for ALL
16 images, so BN batch stats are fully local and NO collective is needed
(a collective would couple every core's NEFF span to the slowest core's
host->device staging, which dominates single-dispatch time).

Conv: x is shipped once as fp16 (exact products with +-1 weights; rel err
~1e-2 vs 2e-2 budget). 16 images run as 4 partition-lanes (g) x 4 batches.
K=32 (ci) matmuls on 16 concurrent 32x32 PE tiles: rows=32g (image lane),
cols=32j. Each col tile packs 28 row-pairs x 4 channels via zero-padded
weight columns: chain (j,k) has its 4 real weight cols at 4k+m and zeros
elsewhere, so every matmul writes the full 32-col group; the first chain
clears (start=True), later chains accumulate zeros harmlessly. PSUM comes
out dense -> full-width ACT drains, no repack.

Epilogue: bn_stats over the dense y_raw, cross-partition combine via tiny
fp32 sel matmuls, Newton-polished rsqrt, ACT affine + DVE/GPSIMD magic-number
round/clip to bf16 {-2..2}; host multiplies by 0.5 and concatenates the
8 cores' channel groups.
"""
import sys
sys.path.insert(0, "/opt/trn_rl_repo")
import numpy as np
import ml_dtypes
import concourse.bass as bass
import concourse.bacc as bacc
import concourse.tile as tile
from concourse import mybir
from concourse.bass_utils import run_bass_kernel_spmd

F32 = mybir.dt.float32
F16 = mybir.dt.float16
BF16 = mybir.dt.bfloat16

N_CORES = 8
CO_PC = 4         # output channels per core
C = 32
H = W = 224
WP = 226          # padded width
NB = 4            # image batches (4 lanes each)
NS = 4            # 56-row supers per image
SR = 56           # rows per super
SLOTS = 58        # input rows per super window (56 + 2 halo)
MAGIC = 12582912.0  # 1.5 * 2**23 -> fp32 round-to-nearest-even trick
EPS = 1e-5
NTOT = float(16 * H * W)  # elements per channel (all on one core)
HWs = H * W

_cache = {}


def _build_nc(loop_n=1, skip=(), dbg=False):
    nc = bacc.Bacc("TRN2", target_bir_lowering=False, debug=False,
                   num_devices=N_CORES)
    xs_ext = nc.declare_dram_parameter("xs", [16, C, H + 2, WP], F16,
                                       isOutput=False)
    dbg_ext = (nc.declare_dram_parameter("dbg", [128, NB, NS, 4, 448], F32,
                                         isOutput=True) if dbg else None)
    dbg2_ext = (nc.declare_dram_parameter("dbg2", [128, 8], F32,
                                          isOutput=True) if dbg == 2 else None)
    s_ext = nc.declare_dram_parameter("s", [128, 9, 7, 32], F16, isOutput=False)
    sel1_ext = nc.declare_dram_parameter("sel1", [128, CO_PC], F32,
                                         isOutput=False)
    sel2_ext = nc.declare_dram_parameter("sel2", [CO_PC, 128], F32,
                                         isOutput=False)
    g_ext = nc.declare_dram_parameter("g", [CO_PC, 1], F32, isOutput=False)
    b_ext = nc.declare_dram_parameter("b", [CO_PC, 1], F32, isOutput=False)
    y_ext = nc.declare_dram_parameter("y", [16, CO_PC, H, W], BF16,
                                      isOutput=True)

    with tile.TileContext(nc) as tc:
        with (
            tc.tile_pool(name="big", bufs=1) as big,
            tc.tile_pool(name="small", bufs=1) as small,
            tc.tile_pool(name="ph2", bufs=2) as ph2,
            tc.tile_pool(name="psum", bufs=1, space="PSUM") as psum,
        ):
            # x super chunk: partition p = 32g + ci ; free = (slot, WP)
            xb = [big.tile([128, SLOTS, WP], F16, name=f"xb{i}", tag=f"x{i}")
                  for i in range(2)]
            # dense conv out: partition p = 32j + 4k + m ; free=(b, s, g, i*w)
            y_raw = big.tile([128, NB, NS, 4, 448], F32)
            s_sb = small.tile([128, 9, 7, 32], F16)
            stats_buf = small.tile([128, 56, 6], F32)
            sel1_sb = small.tile([128, CO_PC], F32)
            sel2_sb = small.tile([CO_PC, 128], F32)
            g_sb = small.tile([CO_PC, 1], F32)
            b_sb = small.tile([CO_PC, 1], F32)
            stats_sq = small.tile([128, 2], F32)
            msq_scr = small.tile([128, 112], F32)
            red = small.tile([128, 4], F32)
            t4 = small.tile([CO_PC, 2], F32)
            fin = small.tile([CO_PC, 8], F32)
            sb4 = small.tile([CO_PC, 2], F32)
            ab128 = small.tile([128, 2], F32)

            psum_t = psum.tile([128, 8, 512], F32)

            nc.vector.memset(stats_buf[:], 0.0)
            nc.sync.dma_start(out=s_sb[:], in_=s_ext[:])
            nc.sync.dma_start(out=sel1_sb[:], in_=sel1_ext[:])
            nc.sync.dma_start(out=sel2_sb[:], in_=sel2_ext[:])
            nc.sync.dma_start(out=g_sb[:], in_=g_ext[:])
            nc.sync.dma_start(out=b_sb[:], in_=b_ext[:])

            pfull = psum_t[:]
            pstride = pfull.ap[0][0]

            import contextlib
            loop_cm = (tc.For_i(0, loop_n, 1) if loop_n > 1
                       else contextlib.nullcontext())
            with loop_cm:
                _body(nc, tc, locals())
    nc.compile()
    return nc


def _body(nc, tc, env):
    xb = env["xb"]
    y_raw, s_sb = env["y_raw"], env["s_sb"]
    stats_buf, sel1_sb, sel2_sb = (env["stats_buf"], env["sel1_sb"],
                                   env["sel2_sb"])
    g_sb, b_sb = env["g_sb"], env["b_sb"]
    stats_sq, msq_scr, red = env["stats_sq"], env["msq_scr"], env["red"]
    t4, fin, sb4, ab128 = env["t4"], env["fin"], env["sb4"], env["ab128"]
    psum_t, ph2 = env["psum_t"], env["ph2"]
    y_ext, xs_ext = env["y_ext"], env["xs_ext"]
    pfull, pstride = env["pfull"], env["pstride"]
    skip = env["skip"]

    xap = xs_ext.ap()
    yap = y_ext.ap()

    # ---- phase 1: conv per (batch, super) ----
    PP = (H + 2) * WP  # padded image size
    for b in range(NB):
        for s in range(NS):
            idx = b * NS + s
            x_c = xb[idx % 2]
            # padded input rows 56s .. 56s+58 (= image rows 56s-1 .. 56s+57)
            if "xdma" not in skip:
                src = bass.AP(
                    tensor=xap.tensor,
                    offset=xap.offset + 4 * b * C * PP + 56 * s * WP,
                    ap=[[C * PP, 4], [PP, C], [1, SLOTS * WP]])
                nc.sync.dma_start(
                    out=x_c.rearrange("p r w -> p (r w)"), in_=src)
            xv = x_c.rearrange("p r w -> p (r w)")
            bank0 = 4 * (idx % 2)
            for k in range(7 if "mm" not in skip else 0):
                for t in range(9):
                    kh, kw = divmod(t, 3)
                    for j in range(4):
                        off = (2 * (7 * j + k) + kh) * WP + kw
                        for g in range(4):
                            nc.tensor.matmul(
                                psum_t[32 * j:32 * j + 32, bank0 + g, 0:450],
                                s_sb[32 * g:32 * g + 32, t, k, :],
                                xv[32 * g:32 * g + 32, off:off + 450],
                                start=(k == 0 and t == 0),
                                stop=(k == 6 and t == 8),
                                tile_position=(32 * g, 32 * j))
            if "epi" in skip:
                continue
            for g in range(4):
                src = bass.AP(
                    tensor=pfull.tensor,
                    offset=pfull.offset + (bank0 + g) * 512,
                    ap=[[pstride, 124], [226, 2], [1, 224]])
                nc.scalar.copy(y_raw[0:124, b, s, g, :], src)

    if env.get("dbg_ext") is not None:
        nc.sync.dma_start(out=env["dbg_ext"].ap(), in_=y_raw[:])

    if "stats" in skip:
        return

    # ---- bulk bn_stats over the dense accumulator (56 x 512-el chunks) ----
    yflat = y_raw.rearrange("p a b c w -> p (a b c w)")
    for i in range(56):
        nc.vector.bn_stats(out=stats_buf[0:124, i, :],
                           in_=yflat[0:124, 512 * i:512 * i + 512])

    # ---- combine bn_stats chunks -> per-partition (sum, sumsq) [128,2] ----
    stats_fl = stats_buf.rearrange("p s (e t) -> p (s e) t", e=2, t=3)
    means = stats_fl[:, :, 1]
    ctv = stats_fl[:, :, 2]
    nc.vector.tensor_reduce(red[:, 0:1], means, mybir.AxisListType.X,
                            mybir.AluOpType.add)
    nc.vector.tensor_tensor(msq_scr[:], means, means, mybir.AluOpType.mult)
    nc.vector.tensor_reduce(red[:, 1:2], msq_scr[:], mybir.AxisListType.X,
                            mybir.AluOpType.add)
    nc.vector.tensor_reduce(red[:, 2:3], ctv, mybir.AxisListType.X,
                            mybir.AluOpType.add)
    nc.vector.tensor_scalar_mul(stats_sq[:, 0:1], red[:, 0:1], 256.0)
    nc.vector.tensor_scalar_mul(red[:, 3:4], red[:, 1:2], 256.0)
    nc.vector.tensor_tensor(stats_sq[:, 1:2], red[:, 3:4], red[:, 2:3],
                            mybir.AluOpType.add)

    # ---- combine (j,k) lanes: [128,2] -> [4,2] via PE ----
    nc.tensor.matmul(psum_t[0:CO_PC, 0, 0:2], sel1_sb[:], stats_sq[:],
                     start=True, stop=True)
    nc.scalar.copy(t4[:], psum_t[0:CO_PC, 0, 0:2])

    # ---- finalize per-channel scale/shift on partitions 0..3 ----
    mean = fin[:, 0:1]
    msqm = fin[:, 1:2]
    v = fin[:, 2:3]
    rec = fin[:, 3:4]
    a_ = fin[:, 4:5]
    bq = fin[:, 5:6]
    cq = fin[:, 6:7]
    sc = fin[:, 7:8]
    inv_n = float(np.float32(1.0) / np.float32(NTOT))
    nc.vector.tensor_scalar_mul(mean, t4[:, 0:1], inv_n)
    nc.vector.tensor_scalar_mul(msqm, t4[:, 1:2], inv_n)
    nc.vector.tensor_tensor(v, mean, mean, mybir.AluOpType.mult)
    nc.vector.tensor_tensor(v, msqm, v, mybir.AluOpType.subtract)
    nc.vector.tensor_scalar_add(v, v, EPS)
    nc.scalar.activation(rec, v, mybir.ActivationFunctionType.Sqrt)
    nc.vector.reciprocal(rec, rec)
    for _ in range(2):  # Newton polish: rec *= 1.5 - 0.5*v*rec^2
        nc.vector.tensor_tensor(a_, rec, rec, mybir.AluOpType.mult)
        nc.vector.tensor_tensor(bq, v, a_, mybir.AluOpType.mult)
        nc.vector.tensor_scalar(cq, bq, -0.5, 1.5, mybir.AluOpType.mult,
                                mybir.AluOpType.add)
        nc.vector.tensor_tensor(rec, rec, cq, mybir.AluOpType.mult)
    nc.vector.tensor_tensor(sc, g_sb[:], rec, mybir.AluOpType.mult)
    nc.vector.tensor_scalar_mul(sb4[:, 0:1], sc, 2.0)
    nc.vector.tensor_tensor(a_, mean, sc, mybir.AluOpType.mult)
    nc.vector.tensor_tensor(bq, b_sb[:], a_, mybir.AluOpType.subtract)
    nc.vector.tensor_scalar_mul(sb4[:, 1:2], bq, 2.0)

    # broadcast [4,2] -> [128,2]
    nc.tensor.matmul(psum_t[:, 1, 0:2], sel2_sb[:], sb4[:],
                     start=True, stop=True)
    nc.scalar.copy(ab128[:], psum_t[:, 1, 0:2])

    if env.get("dbg2_ext") is not None:
        dscr = env["small"].tile([128, 8], F32, name="dscr")
        nc.vector.memset(dscr[:], 0.0)
        nc.scalar.copy(dscr[:, 0:2], stats_sq[:])
        nc.scalar.copy(dscr[:, 2:4], ab128[:])
        nc.scalar.copy(dscr[0:CO_PC, 4:6], t4[:])
        nc.scalar.copy(dscr[0:CO_PC, 6:8], sb4[:])
        nc.sync.dma_start(out=env["dbg2_ext"].ap(), in_=dscr[:])

    # ---- phase 2: normalize + quantize + writeback, per (b, s) chunk ----
    for b in range(NB):
        for s in range(NS):
            if "ph2" in skip:
                break
            zin = y_raw[0:124, b, s].rearrange("p g w -> p (g w)")
            u = ph2.tile([128, 4 * 448], F32, tag="u")
            nc.scalar.activation(u[0:124], zin,
                                 mybir.ActivationFunctionType.Identity,
                                 bias=ab128[0:124, 1:2],
                                 scale=ab128[0:124, 0:1])
            u2 = ph2.tile([128, 4 * 448], F32, tag="u2")
            nc.vector.tensor_scalar(u2[0:124], u[0:124], MAGIC, MAGIC + 2.0,
                                    mybir.AluOpType.add,
                                    mybir.AluOpType.min)
            o = ph2.tile([128, 4 * 448], BF16, tag="o")
            nc.vector.tensor_scalar(o[0:124], u2[0:124], MAGIC - 2.0, MAGIC,
                                    mybir.AluOpType.max,
                                    mybir.AluOpType.subtract)
            ov = o
            for g in range(4):
                for j in range(4):
                    dst = bass.AP(
                        tensor=yap.tensor,
                        offset=(yap.offset + (4 * b + g) * CO_PC * HWs
                                + (56 * s + 14 * j) * W),
                        ap=[[2 * W, 7], [HWs, CO_PC], [1, 448]])
                    nc.scalar.dma_start(
                        out=dst,
                        in_=ov[32 * j:32 * j + 28, g * 448:(g + 1) * 448])


def _get_nc(**kw):
    kw.pop("collective", None)  # compat with old test harness
    key = tuple(sorted((k, tuple(v) if isinstance(v, (list, tuple, set)) else v)
                       for k, v in kw.items()))
    if key not in _cache:
        _cache[key] = _build_nc(**kw)
    return _cache[key]


def _host_consts(weight):
    w_bin = np.where(np.asarray(weight, dtype=np.float32) >= 0, 1.0,
                     -1.0).astype(np.float32)
    # sel over partitions p = 32j + col, col = 4k + m valid when col < 28
    p = np.arange(128)
    col = p % 32
    valid = col < 28
    m_of_p = col % 4
    sel1 = ((m_of_p[:, None] == np.arange(CO_PC)[None, :]) & valid[:, None]
            ).astype(np.float32)
    sel2 = np.ascontiguousarray(sel1.T)
    return w_bin, sel1, sel2


def _stage_s(w_bin, c):
    # S[32g+ci, t, k, 4k+m] = w_bin[4c+m, ci, t]
    wt = np.transpose(w_bin[4 * c:4 * c + 4].reshape(CO_PC, C, 9),
                      (1, 2, 0))  # [ci, t, m]
    s32 = np.zeros((C, 9, 7, 32), dtype=ml_dtypes.float16
                   if hasattr(ml_dtypes, "float16") else np.float16)
    for k in range(7):
        s32[:, :, k, 4 * k:4 * k + 4] = wt
    return np.tile(s32, (4, 1, 1, 1))  # [128, 9, 7, 32]


def make_in_maps(x, weight, gamma, beta):
    xq = np.asarray(x, dtype=np.float32).astype(np.float16)
    xs = np.zeros((16, C, H + 2, WP), dtype=np.float16)
    xs[:, :, 1:225, 1:225] = xq
    w_bin, sel1, sel2 = _host_consts(weight)
    gam = np.asarray(gamma, dtype=np.float32)
    bet = np.asarray(beta, dtype=np.float32)
    in_maps = []
    for c in range(N_CORES):
        in_maps.append({
            "xs": xs, "s": _stage_s(w_bin, c), "sel1": sel1, "sel2": sel2,
            "g": gam[4 * c:4 * c + 4].reshape(CO_PC, 1),
            "b": bet[4 * c:4 * c + 4].reshape(CO_PC, 1)})
    return in_maps


def kernel(x, weight, gamma, beta):
    nc = _get_nc()
    in_maps = make_in_maps(x, weight, gamma, beta)
    res = run_bass_kernel_spmd(nc, in_maps, list(range(N_CORES)))
    out = np.concatenate([res.results[c]["y"] for c in range(N_CORES)], axis=1)
    return out.astype(np.float32) * 0.5


# revision 29
# speedup vs baseline: 1.7544x; 1.3570x over previous
"""Trainium2 Bass kernel for BinarizeConv2d block:
   y = round(2*clip(BN(conv3x3(x, sign(w))), -1, 1))/2

Output-channel sharding: each of 8 cores computes 4 output channels for ALL
16 images, so BN batch stats are fully local and NO collective is needed
(a collective would couple every core's NEFF span to the slowest core's
host->device staging, which dominates single-dispatch time).

Conv: x is shipped once as fp16 (exact products with +-1 weights; rel err
~1e-2 vs 2e-2 budget). 16 images run as 4 partition-lanes (g) x 4 batches.
K=32 (ci) matmuls on 16 concurrent 32x32 PE tiles: rows=32g (image lane),
cols=32j. Each col tile packs 28 row-pairs x 4 channels via zero-padded
weight columns: chain (j,k) has its 4 real weight cols at 4k+m and zeros
elsewhere, so every matmul writes the full 32-col group; the first chain
clears (start=True), later chains accumulate zeros harmlessly. PSUM comes
out dense -> full-width ACT drains, no repack.

Epilogue: bn_stats over the dense y_raw, cross-partition combine via tiny
fp32 sel matmuls, Newton-polished rsqrt, ACT affine + DVE/GPSIMD magic-number
round/clip to bf16 {-2..2}; host multiplies by 0.5 and concatenates the
8 cores' channel groups.
"""
import sys
sys.path.insert(0, "/opt/trn_rl_repo")
import numpy as np
import ml_dtypes
import concourse.bass as bass
import concourse.bacc as bacc
import concourse.tile as tile
from concourse import mybir
from concourse.bass_utils import run_bass_kernel_spmd

F32 = mybir.dt.float32
F16 = mybir.dt.float16
BF16 = mybir.dt.bfloat16

N_CORES = 8
CO_PC = 4         # output channels per core
C = 32
H = W = 224
WP = 226          # padded width
NB = 4            # image batches (4 lanes each)
NS = 4            # 56-row supers per image
SR = 56           # rows per super
SLOTS = 58        # input rows per super window (56 + 2 halo)
MAGIC = 12582912.0  # 1.5 * 2**23 -> fp32 round-to-nearest-even trick
EPS = 1e-5
NTOT = float(16 * H * W)  # elements per channel (all on one core)
HWs = H * W

_cache = {}


def _build_nc(loop_n=1, skip=(), dbg=False):
    nc = bacc.Bacc("TRN2", target_bir_lowering=False, debug=False,
                   num_devices=N_CORES)
    xs_ext = nc.declare_dram_parameter("xs", [16, C, H + 2, WP], F16,
                                       isOutput=False)
    dbg_ext = (nc.declare_dram_parameter("dbg", [128, NB, NS, 4, 448], F32,
                                         isOutput=True) if dbg else None)
    dbg2_ext = (nc.declare_dram_parameter("dbg2", [128, 8], F32,
                                          isOutput=True) if dbg == 2 else None)
    s_ext = nc.declare_dram_parameter("s", [128, 9, 124], F16, isOutput=False)
    sel1_ext = nc.declare_dram_parameter("sel1", [128, CO_PC], F32,
                                         isOutput=False)
    sel2_ext = nc.declare_dram_parameter("sel2", [CO_PC, 128], F32,
                                         isOutput=False)
    g_ext = nc.declare_dram_parameter("g", [CO_PC, 1], F32, isOutput=False)
    b_ext = nc.declare_dram_parameter("b", [CO_PC, 1], F32, isOutput=False)
    y_ext = nc.declare_dram_parameter("y", [16, CO_PC, H, W], BF16,
                                      isOutput=True)

    with tile.TileContext(nc) as tc:
        with (
            tc.tile_pool(name="big", bufs=1) as big,
            tc.tile_pool(name="small", bufs=1) as small,
            tc.tile_pool(name="ph2", bufs=2) as ph2,
            tc.tile_pool(name="psum", bufs=1, space="PSUM") as psum,
        ):
            # x super chunk: partition p = 32g + ci ; free = (slot, WP)
            xb = [big.tile([128, SLOTS, WP], F16, name=f"xb{i}", tag=f"x{i}")
                  for i in range(2)]
            # dense conv out: partition p = 64J + 4k + m ; free=(b, s, g, i*w)
            y_raw = big.tile([128, NB, NS, 4, 448], F32)
            s_sb = small.tile([128, 9, 124], F16)
            stats_buf = small.tile([128, 56, 6], F32)
            sel1_sb = small.tile([128, CO_PC], F32)
            sel2_sb = small.tile([CO_PC, 128], F32)
            g_sb = small.tile([CO_PC, 1], F32)
            b_sb = small.tile([CO_PC, 1], F32)
            stats_sq = small.tile([128, 2], F32)
            msq_scr = small.tile([128, 112], F32)
            red = small.tile([128, 4], F32)
            t4 = small.tile([CO_PC, 2], F32)
            fin = small.tile([CO_PC, 8], F32)
            sb4 = small.tile([CO_PC, 2], F32)
            ab128 = small.tile([128, 2], F32)

            psum_t = psum.tile([128, 8, 512], F32)

            nc.vector.memset(stats_buf[:], 0.0)
            nc.sync.dma_start(out=s_sb[:], in_=s_ext[:])
            nc.sync.dma_start(out=sel1_sb[:], in_=sel1_ext[:])
            nc.sync.dma_start(out=sel2_sb[:], in_=sel2_ext[:])
            nc.sync.dma_start(out=g_sb[:], in_=g_ext[:])
            nc.sync.dma_start(out=b_sb[:], in_=b_ext[:])

            pfull = psum_t[:]
            pstride = pfull.ap[0][0]

            import contextlib
            loop_cm = (tc.For_i(0, loop_n, 1) if loop_n > 1
                       else contextlib.nullcontext())
            with loop_cm:
                _body(nc, tc, locals())
    nc.compile()
    return nc


def _body(nc, tc, env):
    xb = env["xb"]
    y_raw, s_sb = env["y_raw"], env["s_sb"]
    stats_buf, sel1_sb, sel2_sb = (env["stats_buf"], env["sel1_sb"],
                                   env["sel2_sb"])
    g_sb, b_sb = env["g_sb"], env["b_sb"]
    stats_sq, msq_scr, red = env["stats_sq"], env["msq_scr"], env["red"]
    t4, fin, sb4, ab128 = env["t4"], env["fin"], env["sb4"], env["ab128"]
    psum_t, ph2 = env["psum_t"], env["ph2"]
    y_ext, xs_ext = env["y_ext"], env["xs_ext"]
    pfull, pstride = env["pfull"], env["pstride"]
    skip = env["skip"]

    xap = xs_ext.ap()
    yap = y_ext.ap()

    # ---- phase 1: conv per (batch, super) ----
    PP = (H + 2) * WP  # padded image size
    for b in range(NB):
        for s in range(NS):
            idx = b * NS + s
            x_c = xb[idx % 2]
            # padded input rows 56s .. 56s+58 (= image rows 56s-1 .. 56s+57)
            if "xdma" not in skip:
                src = bass.AP(
                    tensor=xap.tensor,
                    offset=xap.offset + 4 * b * C * PP + 56 * s * WP,
                    ap=[[C * PP, 4], [PP, C], [1, SLOTS * WP]])
                nc.sync.dma_start(
                    out=x_c.rearrange("p r w -> p (r w)"), in_=src)
            xv = x_c.rearrange("p r w -> p (r w)")
            bank0 = 4 * (idx % 2)
            for k in range(14 if "mm" not in skip else 0):
                for t in range(9):
                    kh, kw = divmod(t, 3)
                    for J in range(2):
                        off = (2 * (14 * J + k) + kh) * WP + kw
                        for g in range(4):
                            nc.tensor.matmul(
                                psum_t[64 * J:64 * J + 64, bank0 + g, 0:450],
                                s_sb[32 * g:32 * g + 32, t,
                                     60 - 4 * k:124 - 4 * k],
                                xv[32 * g:32 * g + 32, off:off + 450],
                                start=(k == 0 and t == 0),
                                stop=(k == 13 and t == 8),
                                tile_position=(32 * g, 64 * J))
            if "epi" in skip:
                continue
            for g in range(4):
                src = bass.AP(
                    tensor=pfull.tensor,
                    offset=pfull.offset + (bank0 + g) * 512,
                    ap=[[pstride, 128], [226, 2], [1, 224]])
                nc.scalar.copy(y_raw[:, b, s, g, :], src)

    if env.get("dbg_ext") is not None:
        nc.sync.dma_start(out=env["dbg_ext"].ap(), in_=y_raw[:])

    if "stats" in skip:
        return

    # ---- bulk bn_stats over the dense accumulator (56 x 512-el chunks) ----
    yflat = y_raw.rearrange("p a b c w -> p (a b c w)")
    for i in range(56):
        nc.vector.bn_stats(out=stats_buf[:, i, :],
                           in_=yflat[:, 512 * i:512 * i + 512])

    # ---- combine bn_stats chunks -> per-partition (sum, sumsq) [128,2] ----
    stats_fl = stats_buf.rearrange("p s (e t) -> p (s e) t", e=2, t=3)
    means = stats_fl[:, :, 1]
    ctv = stats_fl[:, :, 2]
    nc.vector.tensor_reduce(red[:, 0:1], means, mybir.AxisListType.X,
                            mybir.AluOpType.add)
    nc.vector.tensor_tensor(msq_scr[:], means, means, mybir.AluOpType.mult)
    nc.vector.tensor_reduce(red[:, 1:2], msq_scr[:], mybir.AxisListType.X,
                            mybir.AluOpType.add)
    nc.vector.tensor_reduce(red[:, 2:3], ctv, mybir.AxisListType.X,
                            mybir.AluOpType.add)
    nc.vector.tensor_scalar_mul(stats_sq[:, 0:1], red[:, 0:1], 256.0)
    nc.vector.tensor_scalar_mul(red[:, 3:4], red[:, 1:2], 256.0)
    nc.vector.tensor_tensor(stats_sq[:, 1:2], red[:, 3:4], red[:, 2:3],
                            mybir.AluOpType.add)

    # ---- combine (j,k) lanes: [128,2] -> [4,2] via PE ----
    nc.tensor.matmul(psum_t[0:CO_PC, 0, 0:2], sel1_sb[:], stats_sq[:],
                     start=True, stop=True)
    nc.scalar.copy(t4[:], psum_t[0:CO_PC, 0, 0:2])

    # ---- finalize per-channel scale/shift on partitions 0..3 ----
    mean = fin[:, 0:1]
    msqm = fin[:, 1:2]
    v = fin[:, 2:3]
    rec = fin[:, 3:4]
    a_ = fin[:, 4:5]
    bq = fin[:, 5:6]
    cq = fin[:, 6:7]
    sc = fin[:, 7:8]
    inv_n = float(np.float32(1.0) / np.float32(NTOT))
    nc.vector.tensor_scalar_mul(mean, t4[:, 0:1], inv_n)
    nc.vector.tensor_scalar_mul(msqm, t4[:, 1:2], inv_n)
    nc.vector.tensor_tensor(v, mean, mean, mybir.AluOpType.mult)
    nc.vector.tensor_tensor(v, msqm, v, mybir.AluOpType.subtract)
    nc.vector.tensor_scalar_add(v, v, EPS)
    nc.scalar.activation(rec, v, mybir.ActivationFunctionType.Sqrt)
    nc.vector.reciprocal(rec, rec)
    for _ in range(2):  # Newton polish: rec *= 1.5 - 0.5*v*rec^2
        nc.vector.tensor_tensor(a_, rec, rec, mybir.AluOpType.mult)
        nc.vector.tensor_tensor(bq, v, a_, mybir.AluOpType.mult)
        nc.vector.tensor_scalar(cq, bq, -0.5, 1.5, mybir.AluOpType.mult,
                                mybir.AluOpType.add)
        nc.vector.tensor_tensor(rec, rec, cq, mybir.AluOpType.mult)
    nc.vector.tensor_tensor(sc, g_sb[:], rec, mybir.AluOpType.mult)
    nc.vector.tensor_scalar_mul(sb4[:, 0:1], sc, 2.0)
    nc.vector.tensor_tensor(a_, mean, sc, mybir.AluOpType.mult)
    nc.vector.tensor_tensor(bq, b_sb[:], a_, mybir.AluOpType.subtract)
    nc.vector.tensor_scalar_mul(sb4[:, 1:2], bq, 2.0)

    # broadcast [4,2] -> [128,2]
    nc.tensor.matmul(psum_t[:, 1, 0:2], sel2_sb[:], sb4[:],
                     start=True, stop=True)
    nc.scalar.copy(ab128[:], psum_t[:, 1, 0:2])

    if env.get("dbg2_ext") is not None:
        dscr = env["small"].tile([128, 8], F32, name="dscr")
        nc.vector.memset(dscr[:], 0.0)
        nc.scalar.copy(dscr[:, 0:2], stats_sq[:])
        nc.scalar.copy(dscr[:, 2:4], ab128[:])
        nc.scalar.copy(dscr[0:CO_PC, 4:6], t4[:])
        nc.scalar.copy(dscr[0:CO_PC, 6:8], sb4[:])
        nc.sync.dma_start(out=env["dbg2_ext"].ap(), in_=dscr[:])

    # ---- phase 2: normalize + quantize + writeback, per (b, s) chunk ----
    for b in range(NB):
        for s in range(NS):
            if "ph2" in skip:
                break
            zin = y_raw[:, b, s].rearrange("p g w -> p (g w)")
            u = ph2.tile([128, 4 * 448], F32, tag="u")
            nc.scalar.activation(u[:], zin,
                                 mybir.ActivationFunctionType.Identity,
                                 bias=ab128[:, 1:2],
                                 scale=ab128[:, 0:1])
            u2 = ph2.tile([128, 4 * 448], F32, tag="u2")
            nc.vector.tensor_scalar(u2[:], u[:], MAGIC, MAGIC + 2.0,
                                    mybir.AluOpType.add,
                                    mybir.AluOpType.min)
            o = ph2.tile([128, 4 * 448], BF16, tag="o")
            nc.vector.tensor_scalar(o[:], u2[:], MAGIC - 2.0, MAGIC,
                                    mybir.AluOpType.max,
                                    mybir.AluOpType.subtract)
            for g in range(4):
                for J in range(2):
                    dst = bass.AP(
                        tensor=yap.tensor,
                        offset=(yap.offset + (4 * b + g) * CO_PC * HWs
                                + (56 * s + 28 * J) * W),
                        ap=[[2 * W, 14], [HWs, CO_PC], [1, 448]])
                    nc.sync.dma_start(
                        out=dst,
                        in_=o[64 * J:64 * J + 56, g * 448:(g + 1) * 448])


def _get_nc(**kw):
    kw.pop("collective", None)  # compat with old test harness
    key = tuple(sorted((k, tuple(v) if isinstance(v, (list, tuple, set)) else v)
                       for k, v in kw.items()))
    if key not in _cache:
        _cache[key] = _build_nc(**kw)
    return _cache[key]


def _host_consts(weight):
    w_bin = np.where(np.asarray(weight, dtype=np.float32) >= 0, 1.0,
                     -1.0).astype(np.float32)
    # sel over partitions p = 64J + col, col = 4k + m valid when col < 56
    p = np.arange(128)
    col = p % 64
    valid = col < 56
    m_of_p = col % 4
    sel1 = ((m_of_p[:, None] == np.arange(CO_PC)[None, :]) & valid[:, None]
            ).astype(np.float32)
    sel2 = np.ascontiguousarray(sel1.T)
    return w_bin, sel1, sel2


def _stage_s(w_bin, c):
    # sliding-window zero pad: chain k slices cols [60-4k, 124-4k); the
    # real 32x4 weight block sits at cols 60..63 (local position 4k).
    wt = np.transpose(w_bin[4 * c:4 * c + 4].reshape(CO_PC, C, 9),
                      (1, 2, 0))  # [ci, t, m]
    s32 = np.zeros((C, 9, 124), dtype=np.float16)
    s32[:, :, 60:64] = wt
    return np.tile(s32, (4, 1, 1))  # [128, 9, 124]


def make_in_maps(x, weight, gamma, beta):
    xq = np.asarray(x, dtype=np.float32).astype(np.float16)
    xs = np.zeros((16, C, H + 2, WP), dtype=np.float16)
    xs[:, :, 1:225, 1:225] = xq
    w_bin, sel1, sel2 = _host_consts(weight)
    gam = np.asarray(gamma, dtype=np.float32)
    bet = np.asarray(beta, dtype=np.float32)
    in_maps = []
    for c in range(N_CORES):
        in_maps.append({
            "xs": xs, "s": _stage_s(w_bin, c), "sel1": sel1, "sel2": sel2,
            "g": gam[4 * c:4 * c + 4].reshape(CO_PC, 1),
            "b": bet[4 * c:4 * c + 4].reshape(CO_PC, 1)})
    return in_maps


def kernel(x, weight, gamma, beta):
    nc = _get_nc()
    in_maps = make_in_maps(x, weight, gamma, beta)
    res = run_bass_kernel_spmd(nc, in_maps, list(range(N_CORES)))
    out = np.concatenate([res.results[c]["y"] for c in range(N_CORES)], axis=1)
    return out.astype(np.float32) * 0.5
